# revision 17
# baseline (speedup 1.0000x reference)
"""Causal multi-head self-attention with RoPE on 8 Trainium2 NeuronCores.

Problem: x[2, 2048, 1024] fp32, 16 heads, d_head=64, causal, RoPE(theta=1e4).
Sharding: core = b*4 + g  (b in {0,1} batch, g in {0..3} head-group of 4 heads).
Each core computes out_partial[2048, 1024] = attn(heads of g) @ wo[:, cols_g].T;
host sums the 4 partials per batch.

Per-core kernel (matmul path in bf16, fp32 PSUM accumulation), software-
pipelined over q-chunks of 512 so projection/attention/output-projection
overlap across chunks (Tile gap-fills the PE during the ACT-bound softmax):

  chunk qc: 1) Q/K projections for chunk qc into [d_head, seq] layout
               (2 heads per 128 partitions) with RoPE fused:
               q_rot = A*cosT + P@(A*sinT)  (P = pair-swap sign matrix via one
               PE matmul; the tables are pair-symmetric so P commutes with the
               elementwise sin multiply), V projection for the 4 seq-tiles of
               the chunk into [k partitions, 4*64+ones] layout.
            2) scores_T[k 128, q 512] = K_tile @ Q_chunk on PE (contraction
               d=64; heads of a pair use partition halves 0:64/64:128 so their
               matmuls pack into different PE row groups and run concurrently),
               exp on ACT over kt-PAIRS [128, 1024] (scale=1/8 fused), causal
               0/1-mask multiply on diagonal tiles, attn_aug[65, 512] +=
               V_aug.T @ probs_T accumulated over k tiles (ones column of
               V_aug makes row 64 the softmax denominator for free). The
               accumulator is copied to SBUF immediately to free its PSUM
               bank; normalization uses reciprocal_approx_fast on a
               DRAM-bounce partition broadcast of the denominator row.
            3) out_partial rows of the chunk = attnT.T @ wo_t.
"""

import os
import sys

sys.path.insert(0, "/opt/trn_rl_repo")

import ml_dtypes
import numpy as np

import concourse.bacc as bacc
import concourse.mybir as mybir
from concourse.tile import TileContext

B = 2
S = 2048
DM = 1024
H = 16
DH = 64
HLOC = 4  # heads per core
SC = 512  # q chunk size
NKT = S // 128  # 16 k tiles
NQC = S // SC  # 4 q chunks
P = 128
KO = DM // P  # 8 contraction subtiles for projections
SCALE = 1.0 / 8.0  # 1/sqrt(DH)
THETA = 10000.0

F32 = mybir.dt.float32
BF16 = mybir.dt.bfloat16

_CACHE = {}


def _build_nc():
    nc = bacc.Bacc("TRN2", enable_partition_id=False)
    Exp = mybir.ActivationFunctionType.Exp

    xT = nc.dram_tensor("xT", [DM, S], BF16, kind="ExternalInput")
    wq_t = nc.dram_tensor("wq_t", [DM, 256], BF16, kind="ExternalInput")
    wk_t = nc.dram_tensor("wk_t", [DM, 256], BF16, kind="ExternalInput")
    wv_t = nc.dram_tensor("wv_t", [DM, 256], BF16, kind="ExternalInput")
    wo_t = nc.dram_tensor("wo_t", [256, DM], BF16, kind="ExternalInput")
    cosT = nc.dram_tensor("cosT", [P, S], F32, kind="ExternalInput")
    sinT = nc.dram_tensor("sinT", [P, S], F32, kind="ExternalInput")
    perm = nc.dram_tensor("perm", [P, P], BF16, kind="ExternalInput")
    masks = nc.dram_tensor("masks", [NQC, P, SC], BF16, kind="ExternalInput")
    outp = nc.dram_tensor("out_partial", [S, DM], F32, kind="ExternalOutput")

    with TileContext(nc) as tc:
        with tc.tile_pool(name="sing", bufs=1) as sing, \
             tc.tile_pool(name="wt2", bufs=3) as wt2, \
             tc.tile_pool(name="wpr", bufs=6) as wpr, \
             tc.tile_pool(name="wnr", bufs=3) as wnr, \
             tc.tile_pool(name="wos", bufs=3) as wos, \
             tc.tile_pool(name="pacc", bufs=2, space="PSUM") as pacc, \
             tc.tile_pool(name="pscr", bufs=2, space="PSUM") as pscr, \
             tc.tile_pool(name="patt", bufs=1, space="PSUM") as patt, \
             tc.tile_pool(name="ddr", bufs=4, space="DRAM") as ddr:
            q_rot = sing.tile([P, 2, S], BF16, tag="q_rot")
            k_rot = sing.tile([P, 2, S], BF16, tag="k_rot")
            v_sb = sing.tile([P, NKT, HLOC, 72], BF16, tag="v_sb")
            attnT = sing.tile([P, 2, S], BF16, tag="attnT")

            xT_sb = sing.tile([P, KO, S], BF16, tag="xT_sb")
            xT_ap = xT[:].rearrange("(ko p) s -> p ko s", p=P)
            # chunk-major loads so chunk 0 compute starts early
            for qc in range(NQC):
                for ko in range(KO):
                    nc.sync.dma_start(
                        xT_sb[:, ko, qc * SC:(qc + 1) * SC],
                        xT_ap[:, ko, qc * SC:(qc + 1) * SC],
                    )
            wq_sb = sing.tile([P, KO, 256], BF16, tag="wq_sb")
            wk_sb = sing.tile([P, KO, 256], BF16, tag="wk_sb")
            wv_sb = sing.tile([P, KO, 256], BF16, tag="wv_sb")
            for t, d in ((wq_sb, wq_t), (wk_sb, wk_t), (wv_sb, wv_t)):
                nc.sync.dma_start(
                    t[:], d[:].rearrange("(ko p) m -> p ko m", p=P)
                )
            cos_sb = sing.tile([P, S], F32, tag="cos_sb")
            sin_sb = sing.tile([P, S], F32, tag="sin_sb")
            nc.sync.dma_start(cos_sb[:], cosT[:])
            nc.sync.dma_start(sin_sb[:], sinT[:])
            perm_sb = sing.tile([P, P], BF16, tag="perm_sb")
            nc.sync.dma_start(perm_sb[:], perm[:])
            masks_sb = sing.tile([P, NQC, SC], BF16, tag="masks_sb")
            nc.sync.dma_start(masks_sb[:], masks[:].rearrange("r p f -> p r f"))
            wo_sb = sing.tile([P, 2, DM], BF16, tag="wo_sb")
            nc.sync.dma_start(
                wo_sb[:], wo_t[:].rearrange("(ko p) m -> p ko m", p=P)
            )
            ones_sb = sing.tile([P, 1], F32, tag="ones_sb")
            nc.vector.memset(ones_sb[:], 1.0)
            nc.vector.tensor_copy(
                out=v_sb[:, :, :, 64:65],
                in_=ones_sb[:, None, None, :].to_broadcast((P, NKT, HLOC, 1)),
            )

            out_ap = outp[:].rearrange("(st p) m -> p st m", p=P)

            for qc in range(NQC):
                cs = slice(qc * SC, (qc + 1) * SC)
                # ---- 1) projections for this chunk ----
                for w_sb, dest in ((wk_sb, k_rot), (wq_sb, q_rot)):
                    for hp in range(2):
                        a_ps = pacc.tile([P, SC], F32, tag="pacc",
                                         name="a_ps")
                        for ko in range(KO):
                            nc.tensor.matmul(
                                a_ps[:],
                                lhsT=w_sb[:, ko, hp * P:(hp + 1) * P],
                                rhs=xT_sb[:, ko, cs],
                                start=(ko == 0),
                                stop=(ko == KO - 1),
                            )
                        t2 = wt2.tile([P, SC], BF16, tag="t2")
                        nc.vector.tensor_mul(
                            out=t2[:], in0=a_ps[:], in1=sin_sb[:, cs]
                        )
                        b_ps = pacc.tile([P, SC], F32, tag="pacc",
                                         name="b_ps")
                        nc.tensor.matmul(
                            b_ps[:], lhsT=perm_sb[:], rhs=t2[:],
                            start=True, stop=True,
                        )
                        dsl = dest[:, hp, cs]
                        nc.vector.tensor_mul(
                            out=dsl, in0=a_ps[:], in1=cos_sb[:, cs]
                        )
                        nc.vector.tensor_add(out=dsl, in0=dsl, in1=b_ps[:])
                for st in range(4 * qc, 4 * qc + 4):
                    v_ps = pacc.tile([P, 256], F32, tag="pacc", name="v_ps")
                    for ko in range(KO):
                        nc.tensor.matmul(
                            v_ps[:],
                            lhsT=xT_sb[:, ko, st * P:(st + 1) * P],
                            rhs=wv_sb[:, ko, :],
                            start=(ko == 0),
                            stop=(ko == KO - 1),
                        )
                    nc.vector.tensor_copy(
                        out=v_sb[:, st, :, 0:64],
                        in_=v_ps[:].rearrange("p (h d) -> p h d", d=DH),
                    )

                # ---- 2) attention for the 4 heads on this chunk ----
                nkt_v = 4 * qc + 4
                for hp in range(2):
                    at_ps = [
                        patt.tile([65, SC], F32, tag=f"attn{hh}",
                                  name=f"at_ps{hh}")
                        for hh in range(2)
                    ]
                    for kp in range(nkt_v // 2):
                        s2 = [
                            pscr.tile([P, 2, SC], F32, tag="scores",
                                      name=f"s2_{hh2}")
                            for hh2 in range(2)
                        ]
                        for j in range(2):
                            kt = 2 * kp + j
                            for hh in range(2):
                                hs = slice(hh * 64, (hh + 1) * 64)
                                nc.tensor.matmul(
                                    s2[hh][:, j, :],
                                    lhsT=k_rot[hs, hp, kt * P:(kt + 1) * P],
                                    rhs=q_rot[hs, hp, cs],
                                    start=True,
                                    stop=True,
                                )
                        for hh in range(2):
                            h = 2 * hp + hh
                            pt = wpr.tile([P, 2, SC], BF16, tag="probs")
                            nc.scalar.activation(
                                out=pt[:], in_=s2[hh][:], func=Exp,
                                scale=SCALE,
                            )
                            for j in range(2):
                                r = 2 * kp + j - 4 * qc
                                if r >= 0:
                                    nc.vector.tensor_mul(
                                        out=pt[:, j, :],
                                        in0=pt[:, j, :],
                                        in1=masks_sb[:, r, :],
                                    )
                            for j in range(2):
                                kt = 2 * kp + j
                                nc.tensor.matmul(
                                    at_ps[hh][:],
                                    lhsT=v_sb[:, kt, h, 0:65],
                                    rhs=pt[:, j, :],
                                    start=(kt == 0),
                                    stop=(kt == nkt_v - 1),
                                )
                    for hh in range(2):
                        # free the accumulator bank fast: copy to SBUF, then
                        # normalize from there (denom row 64, attn rows 0:64)
                        asb = wnr.tile([P, SC], F32, tag="asb")
                        nc.vector.tensor_copy(
                            out=asb[0:65, :], in_=at_ps[hh][:]
                        )
                        dr = ddr.tile([1, SC], F32, tag="denr")
                        nc.sync.dma_start(dr[:], asb[64:65, :])
                        den_bc = wnr.tile([64, SC], F32, tag="den_bc")
                        nc.sync.dma_start(
                            den_bc[:], dr[:].partition_broadcast(64)
                        )
                        rbc = wnr.tile([64, SC], F32, tag="rbc")
                        nc.vector.reciprocal_approx_fast(
                            out=rbc[:], in_=den_bc[:]
                        )
                        if hh == 0:
                            nc.vector.tensor_mul(
                                out=attnT[0:64, hp, cs],
                                in0=asb[0:64, :],
                                in1=rbc[:],
                            )
                        else:
                            tmp = wnr.tile([64, SC], BF16, tag="shift")
                            nc.vector.tensor_mul(
                                out=tmp[:], in0=asb[0:64, :], in1=rbc[:]
                            )
                            nc.sync.dma_start(attnT[64:128, hp, cs], tmp[:])

                # ---- 3) output projection for this chunk's seq tiles ----
                for st in range(4 * qc, 4 * qc + 4):
                    o_t = wos.tile([P, DM], F32, tag="ostg")
                    for no in range(2):
                        o_ps = pacc.tile([P, SC], F32, tag="pacc",
                                         name="o_ps")
                        for ko in range(2):
                            nc.tensor.matmul(
                                o_ps[:],
                                lhsT=attnT[:, ko, st * P:(st + 1) * P],
                                rhs=wo_sb[:, ko, no * SC:(no + 1) * SC],
                                start=(ko == 0),
                                stop=(ko == 1),
                            )
                        nc.vector.tensor_copy(
                            out=o_t[:, no * SC:(no + 1) * SC], in_=o_ps[:]
                        )
                    nc.sync.dma_start(out_ap[:, st, :], o_t[:])
    nc.compile()
    return nc


def _host_tables(token_positions):
    pos = np.asarray(token_positions).astype(np.float64)
    freq = 1.0 / (THETA ** (2.0 * np.arange(DH // 2, dtype=np.float64) / DH))
    ang = pos[:, None] * freq[None, :]  # [S, 32]
    cos_f = np.repeat(np.cos(ang), 2, axis=1)  # [S, 64]
    sin_f = np.repeat(np.sin(ang), 2, axis=1)
    cosT = np.ascontiguousarray(
        np.concatenate([cos_f.T, cos_f.T], axis=0)
    ).astype(np.float32)  # [128, S]
    sinT = np.ascontiguousarray(
        np.concatenate([sin_f.T, sin_f.T], axis=0)
    ).astype(np.float32)

    perm = np.zeros((P, P), dtype=ml_dtypes.bfloat16)
    for i in range(P // 2):
        perm[2 * i + 1, 2 * i] = -1.0
        perm[2 * i, 2 * i + 1] = 1.0

    p_idx = np.arange(P)[:, None]
    f_idx = np.arange(SC)[None, :]
    masks = np.stack(
        [
            (f_idx >= p_idx + P * r).astype(ml_dtypes.bfloat16)
            for r in range(NQC)
        ]
    )  # [4, 128, 512]
    return cosT, sinT, perm, masks


_LAST_RESULTS = None


def _bf16(a):
    return np.ascontiguousarray(a).astype(ml_dtypes.bfloat16)


def kernel(x, wq, wk, wv, wo, token_positions):
    global _LAST_RESULTS
    from concourse.bass_utils import run_bass_kernel_spmd

    if "nc" not in _CACHE:
        _CACHE["nc"] = _build_nc()
    nc = _CACHE["nc"]

    x = np.asarray(x, dtype=np.float32)
    wq = np.asarray(wq, dtype=np.float32)
    wk = np.asarray(wk, dtype=np.float32)
    wv = np.asarray(wv, dtype=np.float32)
    wo = np.asarray(wo, dtype=np.float32)
    cosT, sinT, perm, masks = _host_tables(token_positions)

    in_maps = []
    for b in range(B):
        xT_b = _bf16(x[b].T)  # [DM, S]
        for g in range(4):
            rows = slice(g * 256, (g + 1) * 256)
            in_maps.append(
                {
                    "xT": xT_b,
                    "wq_t": _bf16(wq[rows].T),
                    "wk_t": _bf16(wk[rows].T),
                    "wv_t": _bf16(wv[rows].T),
                    "wo_t": _bf16(wo[:, rows].T),
                    "cosT": cosT,
                    "sinT": sinT,
                    "perm": perm,
                    "masks": masks,
                }
            )

    res = run_bass_kernel_spmd(
        nc,
        in_maps,
        core_ids=list(range(8)),
        trace=bool(os.environ.get("BASS_TRACE")),
    )
    _LAST_RESULTS = res
    outs = res.results

    out = np.zeros((B, S, DM), dtype=np.float32)
    for b in range(B):
        for g in range(4):
            out[b] += outs[b * 4 + g]["out_partial"]
    return out


# revision 18
# speedup vs baseline: 1.0708x; 1.0708x over previous
"""Causal multi-head self-attention with RoPE on 8 Trainium2 NeuronCores.

Problem: x[2, 2048, 1024] fp32, 16 heads, d_head=64, causal, RoPE(theta=1e4).
Sharding: core = b*4 + g  (b in {0,1} batch, g in {0..3} head-group of 4 heads).
Each core computes out_partial[2048, 1024] = attn(heads of g) @ wo[:, cols_g].T;
host sums the 4 partials per batch.

Per-core kernel (matmul path in bf16, fp32 PSUM accumulation), software-
pipelined over q-chunks of 512 so projection/attention/output-projection
overlap across chunks (Tile gap-fills the PE during the ACT-bound softmax):

  chunk qc: 1) Q/K projections for chunk qc into [d_head, seq] layout
               (2 heads per 128 partitions) with RoPE fused:
               q_rot = A*cosT + P@(A*sinT)  (P = pair-swap sign matrix via one
               PE matmul; the tables are pair-symmetric so P commutes with the
               elementwise sin multiply). Both RoPE partial products go to
               SBUF temps so each projection holds only one PSUM slot.
               V projection for the chunk's 4 seq-tiles into
               [k partitions, 4*64+ones] layout.
            2) Per k-tile kt: scores_T[k 128, (head-of-pair, q 512)] =
               K_tile @ Q_chunk for both heads of a pair (partition halves
               0:64/64:128 pack into different PE row groups and run
               concurrently), ONE exp on ACT over [128, 1024] covering both
               heads (scale=1/8 fused), causal 0/1-mask multiply on diagonal
               tiles, attn_aug[65, 512] += V_aug.T @ probs_T per head over k
               tiles (the ones column of V_aug makes row 64 the softmax
               denominator for free). Accumulators are copied to SBUF
               immediately to free PSUM; normalization uses
               reciprocal_approx_fast on a DRAM-bounce partition broadcast.
            3) out_partial rows of the chunk = attnT.T @ wo_t.
"""

import os
import sys

sys.path.insert(0, "/opt/trn_rl_repo")

import ml_dtypes
import numpy as np

import concourse.bacc as bacc
import concourse.mybir as mybir
from concourse.tile import TileContext

B = 2
S = 2048
DM = 1024
H = 16
DH = 64
HLOC = 4  # heads per core
SC = 512  # q chunk size
NKT = S // 128  # 16 k tiles
NQC = S // SC  # 4 q chunks
P = 128
KO = DM // P  # 8 contraction subtiles for projections
SCALE = 1.0 / 8.0  # 1/sqrt(DH)
THETA = 10000.0

F32 = mybir.dt.float32
BF16 = mybir.dt.bfloat16

_CACHE = {}


def _build_nc():
    nc = bacc.Bacc("TRN2", enable_partition_id=False)
    Exp = mybir.ActivationFunctionType.Exp

    xT = nc.dram_tensor("xT", [DM, S], BF16, kind="ExternalInput")
    wq_t = nc.dram_tensor("wq_t", [DM, 256], BF16, kind="ExternalInput")
    wk_t = nc.dram_tensor("wk_t", [DM, 256], BF16, kind="ExternalInput")
    wv_t = nc.dram_tensor("wv_t", [DM, 256], BF16, kind="ExternalInput")
    wo_t = nc.dram_tensor("wo_t", [256, DM], BF16, kind="ExternalInput")
    cosT = nc.dram_tensor("cosT", [P, S], F32, kind="ExternalInput")
    sinT = nc.dram_tensor("sinT", [P, S], F32, kind="ExternalInput")
    perm = nc.dram_tensor("perm", [P, P], BF16, kind="ExternalInput")
    masks = nc.dram_tensor("masks", [NQC, P, SC], BF16, kind="ExternalInput")
    outp = nc.dram_tensor("out_partial", [S, DM], F32, kind="ExternalOutput")

    with TileContext(nc) as tc:
        with tc.tile_pool(name="sing", bufs=1) as sing, \
             tc.tile_pool(name="wt2", bufs=4) as wt2, \
             tc.tile_pool(name="wpr", bufs=6) as wpr, \
             tc.tile_pool(name="wnr", bufs=3) as wnr, \
             tc.tile_pool(name="wos", bufs=3) as wos, \
             tc.tile_pool(name="pacc", bufs=2, space="PSUM") as pacc, \
             tc.tile_pool(name="pscr", bufs=2, space="PSUM") as pscr, \
             tc.tile_pool(name="patt", bufs=1, space="PSUM") as patt, \
             tc.tile_pool(name="ddr", bufs=4, space="DRAM") as ddr:
            q_rot = sing.tile([P, 2, S], BF16, tag="q_rot")
            k_rot = sing.tile([P, 2, S], BF16, tag="k_rot")
            v_sb = sing.tile([P, NKT, HLOC, 72], BF16, tag="v_sb")
            attnT = sing.tile([P, 2, S], BF16, tag="attnT")

            # small inputs first so compute can start as soon as possible
            wq_sb = sing.tile([P, KO, 256], BF16, tag="wq_sb")
            wk_sb = sing.tile([P, KO, 256], BF16, tag="wk_sb")
            wv_sb = sing.tile([P, KO, 256], BF16, tag="wv_sb")
            for t, d in ((wq_sb, wq_t), (wk_sb, wk_t), (wv_sb, wv_t)):
                nc.sync.dma_start(
                    t[:], d[:].rearrange("(ko p) m -> p ko m", p=P)
                )
            cos_sb = sing.tile([P, S], F32, tag="cos_sb")
            sin_sb = sing.tile([P, S], F32, tag="sin_sb")
            nc.sync.dma_start(cos_sb[:], cosT[:])
            nc.sync.dma_start(sin_sb[:], sinT[:])
            perm_sb = sing.tile([P, P], BF16, tag="perm_sb")
            nc.sync.dma_start(perm_sb[:], perm[:])
            masks_sb = sing.tile([P, NQC, SC], BF16, tag="masks_sb")
            nc.sync.dma_start(masks_sb[:], masks[:].rearrange("r p f -> p r f"))
            wo_sb = sing.tile([P, 2, DM], BF16, tag="wo_sb")
            nc.sync.dma_start(
                wo_sb[:], wo_t[:].rearrange("(ko p) m -> p ko m", p=P)
            )
            ones_sb = sing.tile([P, 1], F32, tag="ones_sb")
            nc.vector.memset(ones_sb[:], 1.0)
            nc.vector.tensor_copy(
                out=v_sb[:, :, :, 64:65],
                in_=ones_sb[:, None, None, :].to_broadcast((P, NKT, HLOC, 1)),
            )

            xT_sb = sing.tile([P, KO, S], BF16, tag="xT_sb")
            xT_ap = xT[:].rearrange("(ko p) s -> p ko s", p=P)
            # chunk-major loads so chunk 0 compute starts early
            for qc in range(NQC):
                for ko in range(KO):
                    nc.sync.dma_start(
                        xT_sb[:, ko, qc * SC:(qc + 1) * SC],
                        xT_ap[:, ko, qc * SC:(qc + 1) * SC],
                    )

            out_ap = outp[:].rearrange("(st p) m -> p st m", p=P)

            for qc in range(NQC):
                cs = slice(qc * SC, (qc + 1) * SC)
                # ---- 1) projections for this chunk ----
                for w_sb, dest in ((wk_sb, k_rot), (wq_sb, q_rot)):
                    for hp in range(2):
                        a_ps = pacc.tile([P, SC], F32, tag="pacc",
                                         name="a_ps")
                        for ko in range(KO):
                            nc.tensor.matmul(
                                a_ps[:],
                                lhsT=w_sb[:, ko, hp * P:(hp + 1) * P],
                                rhs=xT_sb[:, ko, cs],
                                start=(ko == 0),
                                stop=(ko == KO - 1),
                            )
                        t2 = wt2.tile([P, SC], BF16, tag="t2")
                        nc.vector.tensor_mul(
                            out=t2[:], in0=a_ps[:], in1=sin_sb[:, cs]
                        )
                        t3 = wt2.tile([P, SC], BF16, tag="t3")
                        nc.vector.tensor_mul(
                            out=t3[:], in0=a_ps[:], in1=cos_sb[:, cs]
                        )
                        b_ps = pacc.tile([P, SC], F32, tag="pacc",
                                         name="b_ps")
                        nc.tensor.matmul(
                            b_ps[:], lhsT=perm_sb[:], rhs=t2[:],
                            start=True, stop=True,
                        )
                        dsl = dest[:, hp, cs]
                        nc.vector.tensor_add(out=dsl, in0=t3[:], in1=b_ps[:])
                for st in range(4 * qc, 4 * qc + 4):
                    v_ps = pacc.tile([P, 256], F32, tag="pacc", name="v_ps")
                    for ko in range(KO):
                        nc.tensor.matmul(
                            v_ps[:],
                            lhsT=xT_sb[:, ko, st * P:(st + 1) * P],
                            rhs=wv_sb[:, ko, :],
                            start=(ko == 0),
                            stop=(ko == KO - 1),
                        )
                    nc.vector.tensor_copy(
                        out=v_sb[:, st, :, 0:64],
                        in_=v_ps[:].rearrange("p (h d) -> p h d", d=DH),
                    )

                # ---- 2) attention for the 4 heads on this chunk ----
                nkt_v = 4 * qc + 4
                for hp in range(2):
                    at_ps = [
                        patt.tile([65, SC], F32, tag=f"attn{hh}",
                                  name=f"at_ps{hh}")
                        for hh in range(2)
                    ]
                    for kt in range(nkt_v):
                        # one scores tile per kt holding both heads of the
                        # pair; the two matmuls pack into PE row groups
                        s2 = pscr.tile([P, 2, SC], F32, tag="scores",
                                       name="s2")
                        for hh in range(2):
                            hs = slice(hh * 64, (hh + 1) * 64)
                            nc.tensor.matmul(
                                s2[:, hh, :],
                                lhsT=k_rot[hs, hp, kt * P:(kt + 1) * P],
                                rhs=q_rot[hs, hp, cs],
                                start=True,
                                stop=True,
                            )
                        pt = wpr.tile([P, 2, SC], BF16, tag="probs")
                        nc.scalar.activation(
                            out=pt[:], in_=s2[:], func=Exp, scale=SCALE,
                        )
                        r = kt - 4 * qc
                        if r >= 0:
                            for hh in range(2):
                                nc.vector.tensor_mul(
                                    out=pt[:, hh, :],
                                    in0=pt[:, hh, :],
                                    in1=masks_sb[:, r, :],
                                )
                        for hh in range(2):
                            nc.tensor.matmul(
                                at_ps[hh][:],
                                lhsT=v_sb[:, kt, 2 * hp + hh, 0:65],
                                rhs=pt[:, hh, :],
                                start=(kt == 0),
                                stop=(kt == nkt_v - 1),
                            )
                    for hh in range(2):
                        # free the accumulator bank fast: copy to SBUF, then
                        # normalize from there (denom row 64, attn rows 0:64)
                        asb = wnr.tile([P, SC], F32, tag="asb")
                        nc.vector.tensor_copy(
                            out=asb[0:65, :], in_=at_ps[hh][:]
                        )
                        dr = ddr.tile([1, SC], F32, tag="denr")
                        nc.sync.dma_start(dr[:], asb[64:65, :])
                        den_bc = wnr.tile([64, SC], F32, tag="den_bc")
                        nc.sync.dma_start(
                            den_bc[:], dr[:].partition_broadcast(64)
                        )
                        rbc = wnr.tile([64, SC], F32, tag="rbc")
                        nc.vector.reciprocal_approx_fast(
                            out=rbc[:], in_=den_bc[:]
                        )
                        if hh == 0:
                            nc.vector.tensor_mul(
                                out=attnT[0:64, hp, cs],
                                in0=asb[0:64, :],
                                in1=rbc[:],
                            )
                        else:
                            tmp = wnr.tile([64, SC], BF16, tag="shift")
                            nc.vector.tensor_mul(
                                out=tmp[:], in0=asb[0:64, :], in1=rbc[:]
                            )
                            nc.sync.dma_start(attnT[64:128, hp, cs], tmp[:])

                # ---- 3) output projection for this chunk's seq tiles ----
                for st in range(4 * qc, 4 * qc + 4):
                    o_t = wos.tile([P, DM], F32, tag="ostg")
                    for no in range(2):
                        o_ps = pacc.tile([P, SC], F32, tag="pacc",
                                         name="o_ps")
                        for ko in range(2):
                            nc.tensor.matmul(
                                o_ps[:],
                                lhsT=attnT[:, ko, st * P:(st + 1) * P],
                                rhs=wo_sb[:, ko, no * SC:(no + 1) * SC],
                                start=(ko == 0),
                                stop=(ko == 1),
                            )
                        nc.vector.tensor_copy(
                            out=o_t[:, no * SC:(no + 1) * SC], in_=o_ps[:]
                        )
                    nc.sync.dma_start(out_ap[:, st, :], o_t[:])
    nc.compile()
    return nc


def _host_tables(token_positions):
    pos = np.asarray(token_positions).astype(np.float64)
    freq = 1.0 / (THETA ** (2.0 * np.arange(DH // 2, dtype=np.float64) / DH))
    ang = pos[:, None] * freq[None, :]  # [S, 32]
    cos_f = np.repeat(np.cos(ang), 2, axis=1)  # [S, 64]
    sin_f = np.repeat(np.sin(ang), 2, axis=1)
    cosT = np.ascontiguousarray(
        np.concatenate([cos_f.T, cos_f.T], axis=0)
    ).astype(np.float32)  # [128, S]
    sinT = np.ascontiguousarray(
        np.concatenate([sin_f.T, sin_f.T], axis=0)
    ).astype(np.float32)

    perm = np.zeros((P, P), dtype=ml_dtypes.bfloat16)
    for i in range(P // 2):
        perm[2 * i + 1, 2 * i] = -1.0
        perm[2 * i, 2 * i + 1] = 1.0

    p_idx = np.arange(P)[:, None]
    f_idx = np.arange(SC)[None, :]
    masks = np.stack(
        [
            (f_idx >= p_idx + P * r).astype(ml_dtypes.bfloat16)
            for r in range(NQC)
        ]
    )  # [4, 128, 512]
    return cosT, sinT, perm, masks


_LAST_RESULTS = None


def _bf16(a):
    return np.ascontiguousarray(a).astype(ml_dtypes.bfloat16)


def kernel(x, wq, wk, wv, wo, token_positions):
    global _LAST_RESULTS
    from concourse.bass_utils import run_bass_kernel_spmd

    if "nc" not in _CACHE:
        _CACHE["nc"] = _build_nc()
    nc = _CACHE["nc"]

    x = np.asarray(x, dtype=np.float32)
    wq = np.asarray(wq, dtype=np.float32)
    wk = np.asarray(wk, dtype=np.float32)
    wv = np.asarray(wv, dtype=np.float32)
    wo = np.asarray(wo, dtype=np.float32)
    cosT, sinT, perm, masks = _host_tables(token_positions)

    in_maps = []
    for b in range(B):
        xT_b = _bf16(x[b].T)  # [DM, S]
        for g in range(4):
            rows = slice(g * 256, (g + 1) * 256)
            in_maps.append(
                {
                    "xT": xT_b,
                    "wq_t": _bf16(wq[rows].T),
                    "wk_t": _bf16(wk[rows].T),
                    "wv_t": _bf16(wv[rows].T),
                    "wo_t": _bf16(wo[:, rows].T),
                    "cosT": cosT,
                    "sinT": sinT,
                    "perm": perm,
                    "masks": masks,
                }
            )

    res = run_bass_kernel_spmd(
        nc,
        in_maps,
        core_ids=list(range(8)),
        trace=bool(os.environ.get("BASS_TRACE")),
    )
    _LAST_RESULTS = res
    outs = res.results

    out = np.zeros((B, S, DM), dtype=np.float32)
    for b in range(B):
        for g in range(4):
            out[b] += outs[b * 4 + g]["out_partial"]
    return out
